# revision 29
# baseline (speedup 1.0000x reference)
# Trainium2 Bass kernel for nn_Net_38233798869763 (Mamba-ish net, L=1).
#
# Math (L=1 collapses the reference):
#   - causal depthwise conv over L=1 reduces to xc = xs0*conv_w[:,3] + conv_b
#   - the SSM scan reduces to y_ssm = delta * xs * (Bm . Cm)  (dA multiplies h0=0)
#   so each layer is:
#     rs   = rsqrt(mean(x^2) + eps)                       per batch row
#     xz   = (x * rs) @ (in_proj_w * norm_w * cw_fold).T  [B, 2*DI]
#     xs   = silu(xz[:, :DI] + conv_b);  sz = silu(xz[:, DI:])
#     dbl  = xs @ x_proj_w.T;  dlo, Bm, Cm = split(dbl)
#     delta= softplus(dlo @ dt_w.T + dt_b)
#     s    = sum(Bm * Cm, -1)
#     x   += ((delta * s + D_ssm) * xs * sz) @ out_w.T
#
# Perf design vs the bf16 baseline (614us -> target ~2.5x):
#   * in_proj / out_proj matmuls run fp8e4 DoubleRow (2 contraction rows per
#     PE pass = half the cycles). Weights are scaled per-output-feature into
#     fp8 range on host; the inverse scales ride for free in the ACT-evac
#     `scale` AP (in_proj) / the DVE scalar_tensor_tensor scalar AP (out).
#     Moving operands are cast to fp8 with a pow2 range scale folded into
#     existing constants (rs broadcast gets *CX via an Exp bias; the s
#     broadcast and D_ssm get *CP on host / in a copy scale).
#   * x is pre-normalized once into fp8 (8 DVE ops) instead of scaling all
#     32 in_proj PSUM tiles by rs (linearity trick no longer needed).
#   * dt_b is folded into the dt matmul via an augmented K=65 row of ones,
#     so the softplus chain evacuates bias-free: Exp singles + paired Ln.
#   * y-chain work splits across GpSimd (xs*sz, squares) and DVE
#     (delta*s_bc, fused (m1+dsm)*q -> fp8, fused residual-evac stt).
#   * tiny broadcast matmuls use float32r moving operands (1 cyc/row vs 4).
import numpy as np
import ml_dtypes

B, IN, D, OUT = 4096, 512, 1024, 256
NL, DI, N, DCONV, DTR = 4, 2048, 16, 4, 64
NCORES = 8
BL = B // NCORES          # 512 batch rows per core
KD = D // 128             # 8   k-tiles over D
KIN = IN // 128           # 4   k-tiles over IN
KDI = DI // 128           # 16  k-tiles over DI
JI = 2 * DI // 128        # 32  j-tiles of in_proj output
GJ = 8                    # j-tiles per w_in DMA group
NG = JI // GJ             # 4   groups (2 xs + 2 z)
CX = 16.0                 # fp8 range scale for normalized x
CP = 16.0                 # fp8 range scale for the out_proj moving operand
LNCX = float(np.log(CX))

_cache = {}


def _host_pack(inputs):
    bf = ml_dtypes.bfloat16
    e4 = ml_dtypes.float8_e4m3
    f32 = np.float32

    def t(a):
        return np.ascontiguousarray(a)

    def q8(a):
        return np.clip(a, -240.0, 240.0).astype(e4)

    p = {}
    # proj MLP (bf16)
    p["w_p1"] = t(inputs["pw1"].T.reshape(KIN, 128, D // 2).transpose(1, 0, 2).astype(bf))
    p["b_p1"] = t(inputs["pb1"].reshape(D // 2 // 128, 128).T.astype(f32))
    p["w_p2"] = t(inputs["pw2"].T.reshape(KIN, 128, D).transpose(1, 0, 2).astype(bf))
    p["b_p2"] = t(inputs["pb2"].reshape(KD, 128).T.astype(f32))
    # dense MLP (bf16)
    dw1T = inputs["dw1"].T            # [D, 2D]
    p["w_d1"] = t(np.stack([
        dw1T[:, g * 1024:(g + 1) * 1024].reshape(KD, 128, 1024).transpose(1, 0, 2)
        for g in range(2)
    ]).astype(bf))                    # [2, 128, 8, 1024]
    p["b_d1"] = t(inputs["db1"].reshape(16, 128).T.astype(f32))
    p["w_d2"] = t(inputs["dw2"].T.reshape(16, 128, OUT).transpose(1, 0, 2).astype(bf))
    p["b_d2"] = t(inputs["db2"].reshape(2, 128).T.astype(f32))
    # per-layer mamba params
    for l in range(NL):
        W_in = (inputs["in_proj_w"][l] * inputs["norm_w"][l][None, :]).astype(np.float64)
        W_in = W_in.copy()
        W_in[:DI] *= inputs["conv_w"][l][:, DCONV - 1].astype(np.float64)[:, None]
        # fp8 scales for W_in [2DI, D]: per-feature on the xs half; one
        # shared scale on the z half so paired [128,1024] evacs can use a
        # single per-partition scale AP
        sin = 192.0 / (np.abs(W_in).max(axis=1, keepdims=True) + 1e-30)
        sin[DI:] = 192.0 / (np.abs(W_in[DI:]).max() + 1e-30)
        W8 = q8((W_in * sin).astype(f32))
        WT = W8.T                                                 # [D, 2*DI]
        p[f"w_in{l}"] = t(np.stack([
            WT[:, g * 1024:(g + 1) * 1024].reshape(KD, 128, 1024).transpose(1, 0, 2)
            for g in range(NG)
        ]))                                                       # [4, 128, 8, 1024] fp8
        cin = (1.0 / (sin[:, 0] * CX)).astype(f32)                # [2DI]
        p[f"b_cin{l}"] = t(cin.reshape(JI, 128).T.astype(f32))    # [128, JI]
        p[f"w_xp{l}"] = t(inputs["x_proj_w"][l].T.reshape(KDI, 128, DTR + 2 * N)
                          .transpose(1, 0, 2).astype(bf))         # [128, 16, 96]
        # dt: augmented K=65 (row 64 = dt_b) so the evac needs no bias
        wdt = np.empty((DTR + 1, KDI, 128), f32)
        wdt[:DTR] = inputs["dt_w"][l].T.reshape(DTR, KDI, 128)
        wdt[DTR] = inputs["dt_b"][l].reshape(KDI, 128)
        p[f"w_dt{l}"] = t(wdt.astype(bf))                         # [65, 16, 128]
        # out_proj fp8 with per-feature scales over rows of out_w [D, DI]
        Wo = inputs["out_w"][l].astype(np.float64)
        so = 192.0 / (np.abs(Wo).max(axis=1, keepdims=True) + 1e-30)
        W8o = q8((Wo * so).astype(f32))
        p[f"w_out{l}"] = t(W8o.T.reshape(KDI, 128, D).transpose(1, 0, 2))  # [128,16,1024] fp8
        oinv = (1.0 / (so[:, 0] * CP)).astype(f32)                # [D]
        p[f"b_oinv{l}"] = t(oinv.reshape(KD, 128).T.astype(f32))  # [128, KD]
        p[f"b_cv{l}"] = t(inputs["conv_b"][l].reshape(KDI, 128).T.astype(f32))
        p[f"d_ssm{l}"] = t((CP * inputs["D_ssm"][l]).reshape(KDI, 128).T.astype(f32))
    # input, transposed + per-core sliced: x^T [IN, B] -> [core][128, KIN, BL]
    xT = inputs["x"].T.astype(bf)                                 # [IN, B]
    xc = []
    for c in range(NCORES):
        s = xT[:, c * BL:(c + 1) * BL].reshape(KIN, 128, BL).transpose(1, 0, 2)
        xc.append(t(s))                                           # [128, 4, 512]
    return p, xc


def _patch_act_tables():
    """Steer the ACT table-set chooser so Exp+Ln co-reside (in
    natural_log_exp_and_others) and Tanh lives with Silu; otherwise the
    per-instruction set choice thrashes ACT_TABLE_LOADs (~1.3us each).
    IMPORTANT: the dict ORDER and SIZE must stay identical to act_info.json
    (set ids are positional), so only the function MEMBERSHIP is edited."""
    import concourse.mybir as mybir
    import concourse.bacc as bacc_mod
    if getattr(bacc_mod, "_act_tables_patched", False):
        return
    orig = bacc_mod.get_activation_tables
    AF = mybir.ActivationFunctionType

    def steered(module_arch):
        tabs = orig(module_arch)
        keep = "natural_log_exp_and_others"
        for name, fns in tabs.items():
            if name != keep:
                fns.discard(AF.Exp)
                fns.discard(AF.Ln)
            if name != "silu_and_others":
                fns.discard(AF.Tanh)
        return tabs

    bacc_mod.get_activation_tables = steered
    bacc_mod._act_tables_patched = True


def _build():
    import concourse.tile as tile
    import concourse.mybir as mybir
    from concourse import bacc

    _patch_act_tables()

    dt = mybir.dt
    AF = mybir.ActivationFunctionType
    ALU = mybir.AluOpType
    DR = mybir.MatmulPerfMode.DoubleRow

    nc = bacc.Bacc("TRN2", target_bir_lowering=False, debug=False,
                   num_devices=NCORES)

    def din(name, shape, dtp):
        return nc.dram_tensor(name, shape, dtp, kind="ExternalInput").ap()

    x_in = din("x_in", [128, KIN, BL], dt.bfloat16)
    w_p1 = din("w_p1", [128, KIN, D // 2], dt.bfloat16)
    b_p1 = din("b_p1", [128, KIN], dt.float32)
    w_p2 = din("w_p2", [128, KIN, D], dt.bfloat16)
    b_p2 = din("b_p2", [128, KD], dt.float32)
    w_d1 = din("w_d1", [2, 128, KD, 1024], dt.bfloat16)
    b_d1 = din("b_d1", [128, 16], dt.float32)
    w_d2 = din("w_d2", [128, 16, OUT], dt.bfloat16)
    b_d2 = din("b_d2", [128, 2], dt.float32)
    w_in = [din(f"w_in{l}", [NG, 128, KD, 1024], dt.float8e4) for l in range(NL)]
    b_cin = [din(f"b_cin{l}", [128, JI], dt.float32) for l in range(NL)]
    w_xp = [din(f"w_xp{l}", [128, KDI, DTR + 2 * N], dt.bfloat16) for l in range(NL)]
    w_dt = [din(f"w_dt{l}", [DTR + 1, KDI, 128], dt.bfloat16) for l in range(NL)]
    w_out = [din(f"w_out{l}", [128, KDI, 1024], dt.float8e4) for l in range(NL)]
    b_oinv = [din(f"b_oinv{l}", [128, KD], dt.float32) for l in range(NL)]
    b_cv = [din(f"b_cv{l}", [128, KDI], dt.float32) for l in range(NL)]
    d_ssm = [din(f"d_ssm{l}", [128, KDI], dt.float32) for l in range(NL)]
    out_d = nc.dram_tensor("out", [2, 128, BL], dt.float32, kind="ExternalOutput").ap()

    with tile.TileContext(nc) as tc:
        with (
            tc.tile_pool(name="singles", bufs=1) as sing,
            tc.tile_pool(name="wg", bufs=3) as wgp,
            tc.tile_pool(name="dwg", bufs=2) as dwgp,
            tc.tile_pool(name="wout", bufs=1) as wwp,
            tc.tile_pool(name="tmp", bufs=1) as tmpp,
            tc.tile_pool(name="ps", bufs=1, space="PSUM") as ps,
        ):
            # ---- constants ----
            eps_t = sing.tile([1, 1], dt.float32)
            nc.vector.memset(eps_t[:], 1e-5)
            lncx_t = sing.tile([1, 1], dt.float32)
            nc.vector.memset(lncx_t[:], LNCX)
            ones_bf = sing.tile([128, 1], dt.bfloat16)
            nc.vector.memset(ones_bf[:], 1.0)
            ones16_b = sing.tile([16, 1], dt.bfloat16)
            nc.vector.memset(ones16_b[:], 1.0)
            ones1_b = sing.tile([1, 128], dt.bfloat16)
            nc.vector.memset(ones1_b[:], 1.0)

            # ---- resident small weights / inputs ----
            x_sb = sing.tile([128, KIN, BL], dt.bfloat16)
            nc.sync.dma_start(x_sb[:], x_in)
            wp1_sb = sing.tile([128, KIN, D // 2], dt.bfloat16)
            nc.sync.dma_start(wp1_sb[:], w_p1)
            wp2_sb = sing.tile([128, KIN, D], dt.bfloat16)
            nc.sync.dma_start(wp2_sb[:], w_p2)
            bp1_sb = sing.tile([128, KIN], dt.float32)
            nc.sync.dma_start(bp1_sb[:], b_p1)
            bp2_sb = sing.tile([128, KD], dt.float32)
            nc.sync.dma_start(bp2_sb[:], b_p2)
            bd1_sb = sing.tile([128, 16], dt.float32)
            nc.sync.dma_start(bd1_sb[:], b_d1)
            wd2_sb = sing.tile([128, 16, OUT], dt.bfloat16)
            nc.sync.dma_start(wd2_sb[:], w_d2)
            bd2_sb = sing.tile([128, 2], dt.float32)
            nc.sync.dma_start(bd2_sb[:], b_d2)

            # ---- persistent activations ----
            xT = sing.tile([128, KD, BL], dt.float32)       # residual stream x^T
            x8 = sing.tile([128, KD, BL], dt.float8e4)      # fp8(x * rs * CX)
            sq_bf = sing.tile([128, KD, BL], dt.bfloat16)   # x^2 (bf16 x copy at L3)
            xs_bf = sing.tile([128, KDI, BL], dt.bfloat16)
            sz_bf = sing.tile([128, KDI, BL], dt.bfloat16)  # silu(z); later xs*sz
            delta_bf = sing.tile([128, KDI, BL], dt.bfloat16)
            p8 = sing.tile([128, KDI, BL], dt.float8e4)     # fp8 out_proj moving
            dbl_sb = sing.tile([DTR + 2 * N, BL], dt.float32)
            dlo_t = sing.tile([DTR + 1, BL], dt.bfloat16)   # row 64 = const 1
            nc.vector.memset(dlo_t[DTR:DTR + 1, :], 1.0)
            bm_t = sing.tile([N, BL], dt.float32)
            cm_t = sing.tile([N, BL], dt.float32)
            prod_b = sing.tile([N, BL], dt.bfloat16)
            s_bc = sing.tile([128, BL], dt.bfloat16)
            s_row = sing.tile([1, BL], dt.bfloat16)
            rs_sb = sing.tile([128, BL], dt.float32)
            rs_t = sing.tile([1, BL], dt.bfloat16)
            lnms_t = sing.tile([1, BL], dt.float32)
            out_sb = sing.tile([128, 2, BL], dt.float32)

            _psn = [0]

            def mm_ps(tag="mm", bufs=6, shape=(128, BL)):
                _psn[0] += 1
                return ps.tile(list(shape), dt.float32, tag=tag, bufs=bufs,
                               name=f"ps_{tag}_{_psn[0]}")

            def rms_x8(scope, pssq):
                # rs (with CX folded via Exp bias) -> bcast -> x8; the ssq
                # stats matmuls were interleaved with the out evacs upstream
                with nc.named_scope(scope):
                    nc.scalar.activation(lnms_t[:], pssq[:], AF.Ln,
                                         bias=eps_t[:], scale=1.0 / D)
                    nc.scalar.activation(rs_t[:], lnms_t[:], AF.Exp,
                                         bias=lncx_t[:], scale=-0.5)
                    prbc = mm_ps(tag="small", bufs=2)
                    nc.tensor.matmul(prbc[:], ones1_b[:], rs_t[:],
                                     start=True, stop=True)
                    nc.vector.tensor_copy(rs_sb[:], prbc[:])
                    for k in range(KD):
                        eng = nc.vector if k % 2 == 0 else nc.gpsimd
                        eng.tensor_mul(x8[:, k, :], xT[:, k, :], rs_sb[:])

            # ======== proj MLP: x -> h1 -> xT; squares for L0 rms ========
            with nc.named_scope("proj_mlp"):
                h1_bf = sq_bf                   # h1 borrows sq_bf[:, 0:KIN, :]
                for j in range(KIN):            # h1 j-tiles (D/2 = 512 -> 4)
                    pt = mm_ps()
                    for k in range(KIN):
                        nc.tensor.matmul(pt[:], wp1_sb[:, k, j * 128:(j + 1) * 128],
                                         x_sb[:, k, :],
                                         start=(k == 0), stop=(k == KIN - 1))
                    nc.scalar.activation(h1_bf[:, j, :], pt[:], AF.Tanh,
                                         bias=bp1_sb[:, j:j + 1])
                for j in range(KD):             # h j-tiles (D = 1024 -> 8)
                    pt = mm_ps()
                    for k in range(KIN):
                        nc.tensor.matmul(pt[:], wp2_sb[:, k, j * 128:(j + 1) * 128],
                                         h1_bf[:, k, :],
                                         start=(k == 0), stop=(k == KIN - 1))
                    nc.scalar.activation(xT[:, j, :], pt[:], AF.Identity,
                                         bias=bp2_sb[:, j:j + 1])
                pssq = mm_ps(tag="small", bufs=2, shape=(1, BL))
                for j in range(KD):             # squares for L0 rms (gpsimd)
                    nc.gpsimd.tensor_mul(sq_bf[:, j, :], xT[:, j, :], xT[:, j, :])
                    nc.tensor.matmul(pssq[:], ones_bf[:], sq_bf[:, j, :],
                                     start=(j == 0), stop=(j == KD - 1))

            # ======== mamba layers ========
            for l in range(NL):
                with nc.named_scope(f"L{l}_pre"):
                    wxp = tmpp.tile([128, KDI, DTR + 2 * N], dt.bfloat16, tag="wxp")
                    nc.sync.dma_start(wxp[:], w_xp[l])
                    wdt = tmpp.tile([DTR + 1, KDI, 128], dt.bfloat16, tag="wdt")
                    nc.sync.dma_start(wdt[:], w_dt[l])
                    bcv = tmpp.tile([128, KDI], dt.float32, tag="bcv")
                    nc.sync.dma_start(bcv[:], b_cv[l])
                    cin = tmpp.tile([128, JI], dt.float32, tag="cin")
                    nc.sync.dma_start(cin[:], b_cin[l])
                    oinv = tmpp.tile([128, KD], dt.float32, tag="oinv")
                    nc.sync.dma_start(oinv[:], b_oinv[l])
                    dsm = tmpp.tile([128, KDI], dt.float32, tag="dsm")
                    nc.sync.dma_start(dsm[:], d_ssm[l])
                    wout = wwp.tile([128, KDI, 1024], dt.float8e4, tag="wout")
                    nc.sync.dma_start(wout[:], w_out[l])

                rms_x8(f"L{l}_rms", pssq)

                # --- in_proj fp8 DoubleRow, xs half first (j 0..15) ---
                with nc.named_scope(f"L{l}_inproj"):
                    wgs = []
                    for g in range(NG):
                        wg = wgp.tile([128, KD, 1024], dt.float8e4, tag="wg")
                        nc.sync.dma_start(wg[:], w_in[l][g])
                        wgs.append(wg)
                    for g in range(2):
                        wg = wgs[g]
                        for jj in range(GJ):
                            pt = mm_ps()
                            for t in range(KD // 2):
                                nc.tensor.matmul(
                                    pt[:],
                                    wg[:, 2 * t:2 * t + 2, jj * 128:(jj + 1) * 128],
                                    x8[:, 2 * t:2 * t + 2, :],
                                    start=(t == 0), stop=(t == KD // 2 - 1),
                                    perf_mode=DR)
                            j = g * GJ + jj
                            nc.scalar.activation(xs_bf[:, j, :], pt[:], AF.Silu,
                                                 bias=bcv[:, j:j + 1],
                                                 scale=cin[:, j:j + 1])

                if l == NL - 1:
                    # prefetch dense-MLP weights; DMA overlaps z/dt/y phases
                    dense_wg = []
                    for g in range(2):
                        wgd = dwgp.tile([128, KD, 1024], dt.bfloat16, tag="dwg",
                                        name=f"dense_wg{g}")
                        nc.sync.dma_start(wgd[:], w_d1[g])
                        dense_wg.append(wgd)

                # --- x_proj (needs only xs) -> dlo; s broadcast chain ---
                with nc.named_scope(f"L{l}_xproj"):
                    pdb = mm_ps(tag="small", bufs=2, shape=(DTR + 2 * N, BL))
                    for k in range(KDI):
                        nc.tensor.matmul(pdb[:], wxp[:, k, :], xs_bf[:, k, :],
                                         start=(k == 0), stop=(k == KDI - 1))
                    # dlo straight off PSUM (shortens the dt critical path)
                    nc.vector.tensor_copy(dlo_t[:DTR, :], pdb[:DTR, :])
                    nc.scalar.copy(dbl_sb[DTR:, :], pdb[DTR:, :])
                    nc.sync.dma_start(bm_t[:], dbl_sb[DTR:DTR + N, :])
                    nc.sync.dma_start(cm_t[:], dbl_sb[DTR + N:DTR + 2 * N, :])
                    nc.vector.tensor_mul(prod_b[:], bm_t[:], cm_t[:])
                    psdot = mm_ps(tag="small", bufs=2, shape=(1, BL))
                    nc.tensor.matmul(psdot[:], ones16_b[:], prod_b[:],
                                     start=True, stop=True)
                    nc.scalar.activation(s_row[:], psdot[:], AF.Copy, scale=CP)
                    psbc = mm_ps(tag="small", bufs=2)
                    nc.tensor.matmul(psbc[:], ones1_b[:], s_row[:],
                                     start=True, stop=True)
                    nc.vector.tensor_copy(s_bc[:], psbc[:])

                # --- z half with paired [128,1024] silu evacs; the ACT z/delta
                #     work overlaps the PE z/out GEMMs ---
                with nc.named_scope(f"L{l}_zproj"):
                    for g in range(2, NG):
                        wg = wgs[g]
                        for jj in range(GJ):
                            pt = mm_ps()
                            for t in range(KD // 2):
                                nc.tensor.matmul(
                                    pt[:],
                                    wg[:, 2 * t:2 * t + 2, jj * 128:(jj + 1) * 128],
                                    x8[:, 2 * t:2 * t + 2, :],
                                    start=(t == 0), stop=(t == KD // 2 - 1),
                                    perf_mode=DR)
                            zj = (g - 2) * GJ + jj
                            nc.scalar.activation(sz_bf[:, zj, :], pt[:], AF.Silu,
                                                 scale=cin[:, KDI + zj:KDI + zj + 1])
                            # q = xs*sz on gpsimd (overwrites sz)
                            nc.gpsimd.tensor_mul(sz_bf[:, zj, :],
                                                 xs_bf[:, zj, :],
                                                 sz_bf[:, zj, :])

                # --- dt matmul (K=65 incl bias row); softplus = Ln(Exp(u)+1);
                #     Exp/Ln/m1/p8 interleaved per k-pair so the first out_proj
                #     DR group starts ~2.5us after the first dt psum ---
                with nc.named_scope(f"L{l}_dt"):
                    for j in range(KDI):
                        pt = mm_ps()
                        nc.tensor.matmul(pt[:], wdt[:, j, :], dlo_t[:],
                                         start=True, stop=True)
                        nc.scalar.activation(delta_bf[:, j, :], pt[:], AF.Exp)
                        if j % 2 == 1:
                            nc.scalar.activation(delta_bf[:, j - 1:j + 1, :],
                                                 delta_bf[:, j - 1:j + 1, :],
                                                 AF.Ln, bias=1.0)
                            for k in (j - 1, j):
                                nc.vector.tensor_mul(delta_bf[:, k, :],
                                                     delta_bf[:, k, :], s_bc[:])
                                nc.vector.scalar_tensor_tensor(
                                    p8[:, k, :], delta_bf[:, k, :],
                                    dsm[:, k:k + 1], sz_bf[:, k, :],
                                    ALU.add, ALU.mult)

                # --- out_proj DR over p8; fused residual evac; next-layer
                #     squares alternate engines to avoid one serial chain ---
                with nc.named_scope(f"L{l}_y_out"):
                    if l < NL - 1:
                        pssq = mm_ps(tag="small", bufs=2, shape=(1, BL))
                    pouts = [mm_ps() for _ in range(KD // 2)]
                    for t in range(KDI // 2):
                        for j in range(KD // 2):
                            nc.tensor.matmul(
                                pouts[j][:],
                                wout[:, 2 * t:2 * t + 2, j * 128:(j + 1) * 128],
                                p8[:, 2 * t:2 * t + 2, :],
                                start=(t == 0), stop=(t == KDI // 2 - 1),
                                perf_mode=DR)

                    def evac_out(j, pt):
                        nc.vector.scalar_tensor_tensor(
                            xT[:, j, :], pt[:], oinv[:, j:j + 1], xT[:, j, :],
                            ALU.mult, ALU.add)
                        if l < NL - 1:
                            if j % 2 == 0:
                                nc.gpsimd.tensor_mul(sq_bf[:, j, :], xT[:, j, :],
                                                     xT[:, j, :])
                            else:
                                nc.scalar.activation(sq_bf[:, j, :], xT[:, j, :],
                                                     AF.Square)
                            nc.tensor.matmul(pssq[:], ones_bf[:], sq_bf[:, j, :],
                                             start=(j == 0), stop=(j == KD - 1))
                        else:
                            if j % 2 == 0:
                                nc.scalar.copy(sq_bf[:, j, :], xT[:, j, :])
                            else:
                                nc.vector.tensor_copy(sq_bf[:, j, :], xT[:, j, :])

                    for j in range(KD // 2):
                        evac_out(j, pouts[j])
                    for j in range(KD // 2, KD):
                        pt = mm_ps()
                        for t in range(KDI // 2):
                            nc.tensor.matmul(
                                pt[:],
                                wout[:, 2 * t:2 * t + 2, j * 128:(j + 1) * 128],
                                p8[:, 2 * t:2 * t + 2, :],
                                start=(t == 0), stop=(t == KDI // 2 - 1),
                                perf_mode=DR)
                        evac_out(j, pt)

            # ======== dense MLP (bf16): x -> g1 -> out ========
            # sq_bf holds the bf16 copy of the final residual.
            with nc.named_scope("dense_mlp"):
                for g in range(2):
                    wg = dense_wg[g]
                    for jj in range(GJ):
                        pt = mm_ps()
                        for k in range(KD):
                            nc.tensor.matmul(pt[:], wg[:, k, jj * 128:(jj + 1) * 128],
                                             sq_bf[:, k, :],
                                             start=(k == 0), stop=(k == KD - 1))
                        j = g * GJ + jj
                        nc.scalar.activation(xs_bf[:, j, :], pt[:], AF.Tanh,
                                             bias=bd1_sb[:, j:j + 1])
                for j in range(2):
                    pt = mm_ps()
                    for k in range(16):
                        nc.tensor.matmul(pt[:], wd2_sb[:, k, j * 128:(j + 1) * 128],
                                         xs_bf[:, k, :], start=(k == 0),
                                         stop=(k == 15))
                    nc.scalar.activation(out_sb[:, j, :], pt[:], AF.Tanh,
                                         bias=bd2_sb[:, j:j + 1])
                    nc.gpsimd.dma_start(out_d[j], out_sb[:, j, :])

    nc.compile()
    return nc


def _run(inputs, trace=False, trace_kwargs=None):
    if "nc" not in _cache:
        _cache["nc"] = _build()
    nc = _cache["nc"]
    p, xc = _host_pack(inputs)
    in_maps = []
    for c in range(NCORES):
        m = dict(p)
        m["x_in"] = xc[c]
        in_maps.append(m)

    from concourse.bass_utils import run_bass_kernel_spmd
    kw = {}
    if trace:
        kw.update(trace=True, trace_cores=[0], trace_kwargs=trace_kwargs or {})
    res = run_bass_kernel_spmd(nc, in_maps, core_ids=list(range(NCORES)), **kw)

    # assemble: per core out [2, 128, BL] -> out^T [256, BL] -> [BL, 256]
    full = np.empty((B, OUT), np.float32)
    for c in range(NCORES):
        o = res.results[c]["out"].reshape(OUT, BL)
        full[c * BL:(c + 1) * BL] = o.T
    return full.reshape(-1), res


def kernel(**inputs):
    out, _ = _run(inputs, trace=False)
    return out


# revision 32
# speedup vs baseline: 1.2094x; 1.2094x over previous
# Trainium2 Bass kernel for nn_Net_38233798869763 (Mamba-ish net, L=1).
#
# Math (L=1 collapses the reference):
#   - causal depthwise conv over L=1 reduces to xc = xs0*conv_w[:,3] + conv_b
#   - the SSM scan reduces to y_ssm = delta * xs * (Bm . Cm)  (dA multiplies h0=0)
#   so each layer is:
#     rs   = rsqrt(mean(x^2) + eps)                       per batch row
#     xz   = (x * rs) @ (in_proj_w * norm_w * cw_fold).T  [B, 2*DI]
#     xs   = silu(xz[:, :DI] + conv_b);  sz = silu(xz[:, DI:])
#     dbl  = xs @ x_proj_w.T;  dlo, Bm, Cm = split(dbl)
#     delta= softplus(dlo @ dt_w.T + dt_b)
#     s    = sum(Bm * Cm, -1)
#     x   += ((delta * s + D_ssm) * xs * sz) @ out_w.T
#
# Perf design vs the bf16 baseline (614us -> target ~2.5x):
#   * in_proj / out_proj matmuls run fp8e4 DoubleRow (2 contraction rows per
#     PE pass = half the cycles). Weights are scaled per-output-feature into
#     fp8 range on host; the inverse scales ride for free in the ACT-evac
#     `scale` AP (in_proj) / the DVE scalar_tensor_tensor scalar AP (out).
#     Moving operands are cast to fp8 with a pow2 range scale folded into
#     existing constants (rs broadcast gets *CX via an Exp bias; the s
#     broadcast and D_ssm get *CP on host / in a copy scale).
#   * x is pre-normalized once into fp8 (8 DVE ops) instead of scaling all
#     32 in_proj PSUM tiles by rs (linearity trick no longer needed).
#   * dt_b is folded into the dt matmul via an augmented K=65 row of ones,
#     so the softplus chain evacuates bias-free: Exp singles + paired Ln.
#   * y-chain work splits across GpSimd (xs*sz, squares) and DVE
#     (delta*s_bc, fused (m1+dsm)*q -> fp8, fused residual-evac stt).
#   * tiny broadcast matmuls use float32r moving operands (1 cyc/row vs 4).
import numpy as np
import ml_dtypes

B, IN, D, OUT = 4096, 512, 1024, 256
NL, DI, N, DCONV, DTR = 4, 2048, 16, 4, 64
NCORES = 8
BL = B // NCORES          # 512 batch rows per core
KD = D // 128             # 8   k-tiles over D
KIN = IN // 128           # 4   k-tiles over IN
KDI = DI // 128           # 16  k-tiles over DI
JI = 2 * DI // 128        # 32  j-tiles of in_proj output
GJ = 8                    # j-tiles per w_in DMA group
NG = JI // GJ             # 4   groups (2 xs + 2 z)
CX = 16.0                 # fp8 range scale for normalized x
CP = 16.0                 # fp8 range scale for the out_proj moving operand
LNCX = float(np.log(CX))

_cache = {}


def _host_pack(inputs):
    bf = ml_dtypes.bfloat16
    e4 = ml_dtypes.float8_e4m3
    f32 = np.float32

    def t(a):
        return np.ascontiguousarray(a)

    def q8(a):
        return np.clip(a, -240.0, 240.0).astype(e4)

    p = {}
    # proj MLP (bf16)
    p["w_p1"] = t(inputs["pw1"].T.reshape(KIN, 128, D // 2).transpose(1, 0, 2).astype(bf))
    p["b_p1"] = t(inputs["pb1"].reshape(D // 2 // 128, 128).T.astype(f32))
    p["w_p2"] = t(inputs["pw2"].T.reshape(KIN, 128, D).transpose(1, 0, 2).astype(bf))
    p["b_p2"] = t(inputs["pb2"].reshape(KD, 128).T.astype(f32))
    # dense MLP (bf16)
    dw1T = inputs["dw1"].T            # [D, 2D]
    p["w_d1"] = t(np.stack([
        dw1T[:, g * 1024:(g + 1) * 1024].reshape(KD, 128, 1024).transpose(1, 0, 2)
        for g in range(2)
    ]).astype(bf))                    # [2, 128, 8, 1024]
    p["b_d1"] = t(inputs["db1"].reshape(16, 128).T.astype(f32))
    p["w_d2"] = t(inputs["dw2"].T.reshape(16, 128, OUT).transpose(1, 0, 2).astype(bf))
    p["b_d2"] = t(inputs["db2"].reshape(2, 128).T.astype(f32))
    # per-layer mamba params
    for l in range(NL):
        W_in = (inputs["in_proj_w"][l] * inputs["norm_w"][l][None, :]).astype(np.float64)
        W_in = W_in.copy()
        W_in[:DI] *= inputs["conv_w"][l][:, DCONV - 1].astype(np.float64)[:, None]
        # fp8 scales for W_in [2DI, D]: per-feature on the xs half; one
        # shared scale on the z half so paired [128,1024] evacs can use a
        # single per-partition scale AP
        sin = 192.0 / (np.abs(W_in).max(axis=1, keepdims=True) + 1e-30)
        sin[DI:] = 192.0 / (np.abs(W_in[DI:]).max() + 1e-30)
        W8 = q8((W_in * sin).astype(f32))
        WT = W8.T                                                 # [D, 2*DI]
        p[f"w_in{l}"] = t(np.stack([
            WT[:, g * 1024:(g + 1) * 1024].reshape(KD, 128, 1024).transpose(1, 0, 2)
            for g in range(NG)
        ]))                                                       # [4, 128, 8, 1024] fp8
        cin = (1.0 / (sin[:, 0] * CX)).astype(f32)                # [2DI]
        p[f"b_cin{l}"] = t(cin.reshape(JI, 128).T.astype(f32))    # [128, JI]
        p[f"w_xp{l}"] = t(inputs["x_proj_w"][l].T.reshape(KDI, 128, DTR + 2 * N)
                          .transpose(1, 0, 2).astype(bf))         # [128, 16, 96]
        # dt: augmented K=65 (row 64 = dt_b) so the evac needs no bias
        wdt = np.empty((DTR + 1, KDI, 128), f32)
        wdt[:DTR] = inputs["dt_w"][l].T.reshape(DTR, KDI, 128)
        wdt[DTR] = inputs["dt_b"][l].reshape(KDI, 128)
        p[f"w_dt{l}"] = t(wdt.astype(bf))                         # [65, 16, 128]
        # out_proj fp8 with per-feature scales over rows of out_w [D, DI]
        Wo = inputs["out_w"][l].astype(np.float64)
        so = 192.0 / (np.abs(Wo).max(axis=1, keepdims=True) + 1e-30)
        W8o = q8((Wo * so).astype(f32))
        p[f"w_out{l}"] = t(W8o.T.reshape(KDI, 128, D).transpose(1, 0, 2))  # [128,16,1024] fp8
        oinv = (1.0 / (so[:, 0] * CP)).astype(f32)                # [D]
        p[f"b_oinv{l}"] = t(oinv.reshape(KD, 128).T.astype(f32))  # [128, KD]
        p[f"b_cv{l}"] = t(inputs["conv_b"][l].reshape(KDI, 128).T.astype(f32))
        p[f"d_ssm{l}"] = t((CP * inputs["D_ssm"][l]).reshape(KDI, 128).T.astype(f32))
    # input, transposed + per-core sliced: x^T [IN, B] -> [core][128, KIN, BL]
    xT = inputs["x"].T.astype(bf)                                 # [IN, B]
    xc = []
    for c in range(NCORES):
        s = xT[:, c * BL:(c + 1) * BL].reshape(KIN, 128, BL).transpose(1, 0, 2)
        xc.append(t(s))                                           # [128, 4, 512]
    return p, xc


def _patch_act_tables():
    """Steer the ACT table-set chooser so Exp+Ln co-reside (in
    natural_log_exp_and_others) and Tanh lives with Silu; otherwise the
    per-instruction set choice thrashes ACT_TABLE_LOADs (~1.3us each).
    IMPORTANT: the dict ORDER and SIZE must stay identical to act_info.json
    (set ids are positional), so only the function MEMBERSHIP is edited."""
    import concourse.mybir as mybir
    import concourse.bacc as bacc_mod
    if getattr(bacc_mod, "_act_tables_patched", False):
        return
    orig = bacc_mod.get_activation_tables
    AF = mybir.ActivationFunctionType

    def steered(module_arch):
        tabs = orig(module_arch)
        keep = "natural_log_exp_and_others"
        for name, fns in tabs.items():
            if name != keep:
                fns.discard(AF.Exp)
                fns.discard(AF.Ln)
            if name != "silu_and_others":
                fns.discard(AF.Tanh)
        return tabs

    bacc_mod.get_activation_tables = steered
    bacc_mod._act_tables_patched = True


def _build():
    import concourse.tile as tile
    import concourse.mybir as mybir
    from concourse import bacc

    _patch_act_tables()

    dt = mybir.dt
    AF = mybir.ActivationFunctionType
    ALU = mybir.AluOpType
    DR = mybir.MatmulPerfMode.DoubleRow

    nc = bacc.Bacc("TRN2", target_bir_lowering=False, debug=False,
                   num_devices=NCORES)

    def din(name, shape, dtp):
        return nc.dram_tensor(name, shape, dtp, kind="ExternalInput").ap()

    x_in = din("x_in", [128, KIN, BL], dt.bfloat16)
    w_p1 = din("w_p1", [128, KIN, D // 2], dt.bfloat16)
    b_p1 = din("b_p1", [128, KIN], dt.float32)
    w_p2 = din("w_p2", [128, KIN, D], dt.bfloat16)
    b_p2 = din("b_p2", [128, KD], dt.float32)
    w_d1 = din("w_d1", [2, 128, KD, 1024], dt.bfloat16)
    b_d1 = din("b_d1", [128, 16], dt.float32)
    w_d2 = din("w_d2", [128, 16, OUT], dt.bfloat16)
    b_d2 = din("b_d2", [128, 2], dt.float32)
    w_in = [din(f"w_in{l}", [NG, 128, KD, 1024], dt.float8e4) for l in range(NL)]
    b_cin = [din(f"b_cin{l}", [128, JI], dt.float32) for l in range(NL)]
    w_xp = [din(f"w_xp{l}", [128, KDI, DTR + 2 * N], dt.bfloat16) for l in range(NL)]
    w_dt = [din(f"w_dt{l}", [DTR + 1, KDI, 128], dt.bfloat16) for l in range(NL)]
    w_out = [din(f"w_out{l}", [128, KDI, 1024], dt.float8e4) for l in range(NL)]
    b_oinv = [din(f"b_oinv{l}", [128, KD], dt.float32) for l in range(NL)]
    b_cv = [din(f"b_cv{l}", [128, KDI], dt.float32) for l in range(NL)]
    d_ssm = [din(f"d_ssm{l}", [128, KDI], dt.float32) for l in range(NL)]
    out_d = nc.dram_tensor("out", [2, 128, BL], dt.float32, kind="ExternalOutput").ap()

    with tile.TileContext(nc) as tc:
        with (
            tc.tile_pool(name="singles", bufs=1) as sing,
            tc.tile_pool(name="wg", bufs=3) as wgp,
            tc.tile_pool(name="dwg", bufs=2) as dwgp,
            tc.tile_pool(name="wout", bufs=1) as wwp,
            tc.tile_pool(name="tmp", bufs=1) as tmpp,
            tc.tile_pool(name="ps", bufs=1, space="PSUM") as ps,
        ):
            # ---- constants ----
            eps_t = sing.tile([1, 1], dt.float32)
            nc.vector.memset(eps_t[:], 1e-5)
            lncx_t = sing.tile([1, 1], dt.float32)
            nc.vector.memset(lncx_t[:], LNCX)
            ones_bf = sing.tile([128, 1], dt.bfloat16)
            nc.vector.memset(ones_bf[:], 1.0)
            ones16_b = sing.tile([16, 1], dt.bfloat16)
            nc.vector.memset(ones16_b[:], 1.0)
            ones1_b = sing.tile([1, 128], dt.bfloat16)
            nc.vector.memset(ones1_b[:], 1.0)

            # ---- resident small weights / inputs ----
            x_sb = sing.tile([128, KIN, BL], dt.bfloat16)
            nc.sync.dma_start(x_sb[:], x_in)
            wp1_sb = sing.tile([128, KIN, D // 2], dt.bfloat16)
            nc.sync.dma_start(wp1_sb[:], w_p1)
            wp2_sb = sing.tile([128, KIN, D], dt.bfloat16)
            nc.sync.dma_start(wp2_sb[:], w_p2)
            bp1_sb = sing.tile([128, KIN], dt.float32)
            nc.sync.dma_start(bp1_sb[:], b_p1)
            bp2_sb = sing.tile([128, KD], dt.float32)
            nc.sync.dma_start(bp2_sb[:], b_p2)
            bd1_sb = sing.tile([128, 16], dt.float32)
            nc.sync.dma_start(bd1_sb[:], b_d1)
            wd2_sb = sing.tile([128, 16, OUT], dt.bfloat16)
            nc.sync.dma_start(wd2_sb[:], w_d2)
            bd2_sb = sing.tile([128, 2], dt.float32)
            nc.sync.dma_start(bd2_sb[:], b_d2)

            # ---- persistent activations ----
            xT = sing.tile([128, KD, BL], dt.float32)       # residual stream x^T
            x8 = sing.tile([128, KD, BL], dt.float8e4)      # fp8(x * rs * CX)
            sq_bf = sing.tile([128, KD, BL], dt.bfloat16)   # x^2 (bf16 x copy at L3)
            xs_bf = sing.tile([128, KDI, BL], dt.bfloat16)
            sz_bf = sing.tile([128, KDI, BL], dt.bfloat16)  # silu(z); later xs*sz
            delta_bf = sing.tile([128, KDI, BL], dt.bfloat16)
            p8 = sing.tile([128, KDI, BL], dt.float8e4)     # fp8 out_proj moving
            dbl_sb = sing.tile([DTR + 2 * N, BL], dt.float32)
            dlo_t = sing.tile([DTR + 1, BL], dt.bfloat16)   # row 64 = const 1
            nc.vector.memset(dlo_t[DTR:DTR + 1, :], 1.0)
            bm_t = sing.tile([N, BL], dt.float32)
            cm_t = sing.tile([N, BL], dt.float32)
            prod_b = sing.tile([N, BL], dt.bfloat16)
            s_bc = sing.tile([128, BL], dt.bfloat16)
            s_row = sing.tile([1, BL], dt.bfloat16)
            rs_sb = sing.tile([128, BL], dt.float32)
            rs_t = sing.tile([1, BL], dt.bfloat16)
            lnms_t = sing.tile([1, BL], dt.float32)
            out_sb = sing.tile([128, 2, BL], dt.float32)

            _psn = [0]

            def mm_ps(tag="mm", bufs=6, shape=(128, BL)):
                _psn[0] += 1
                return ps.tile(list(shape), dt.float32, tag=tag, bufs=bufs,
                               name=f"ps_{tag}_{_psn[0]}")

            def rms_x8(scope, pssq):
                # rs (with CX folded via Exp bias) -> bcast -> x8; the ssq
                # stats matmuls were interleaved with the out evacs upstream
                with nc.named_scope(scope):
                    nc.scalar.activation(lnms_t[:], pssq[:], AF.Ln,
                                         bias=eps_t[:], scale=1.0 / D)
                    nc.scalar.activation(rs_t[:], lnms_t[:], AF.Exp,
                                         bias=lncx_t[:], scale=-0.5)
                    prbc = mm_ps(tag="small", bufs=2)
                    nc.tensor.matmul(prbc[:], ones1_b[:], rs_t[:],
                                     start=True, stop=True)
                    nc.vector.tensor_copy(rs_sb[:], prbc[:])
                    for k in range(KD):
                        nc.vector.tensor_mul(x8[:, k, :], xT[:, k, :], rs_sb[:])

            # ======== proj MLP: x -> h1 -> xT; squares for L0 rms ========
            with nc.named_scope("proj_mlp"):
                h1_bf = sq_bf                   # h1 borrows sq_bf[:, 0:KIN, :]
                for j in range(KIN):            # h1 j-tiles (D/2 = 512 -> 4)
                    pt = mm_ps()
                    for k in range(KIN):
                        nc.tensor.matmul(pt[:], wp1_sb[:, k, j * 128:(j + 1) * 128],
                                         x_sb[:, k, :],
                                         start=(k == 0), stop=(k == KIN - 1))
                    nc.scalar.activation(h1_bf[:, j, :], pt[:], AF.Tanh,
                                         bias=bp1_sb[:, j:j + 1])
                for j in range(KD):             # h j-tiles (D = 1024 -> 8)
                    pt = mm_ps()
                    for k in range(KIN):
                        nc.tensor.matmul(pt[:], wp2_sb[:, k, j * 128:(j + 1) * 128],
                                         h1_bf[:, k, :],
                                         start=(k == 0), stop=(k == KIN - 1))
                    nc.scalar.activation(xT[:, j, :], pt[:], AF.Identity,
                                         bias=bp2_sb[:, j:j + 1])
                pssq = mm_ps(tag="small", bufs=2, shape=(1, BL))
                for j in range(KD):             # squares for L0 rms (gpsimd)
                    nc.gpsimd.tensor_mul(sq_bf[:, j, :], xT[:, j, :], xT[:, j, :])
                    nc.tensor.matmul(pssq[:], ones_bf[:], sq_bf[:, j, :],
                                     start=(j == 0), stop=(j == KD - 1))

            # ======== mamba layers ========
            for l in range(NL):
                with nc.named_scope(f"L{l}_pre"):
                    wxp = tmpp.tile([128, KDI, DTR + 2 * N], dt.bfloat16, tag="wxp")
                    nc.sync.dma_start(wxp[:], w_xp[l])
                    wdt = tmpp.tile([DTR + 1, KDI, 128], dt.bfloat16, tag="wdt")
                    nc.sync.dma_start(wdt[:], w_dt[l])
                    bcv = tmpp.tile([128, KDI], dt.float32, tag="bcv")
                    nc.sync.dma_start(bcv[:], b_cv[l])
                    cin = tmpp.tile([128, JI], dt.float32, tag="cin")
                    nc.sync.dma_start(cin[:], b_cin[l])
                    oinv = tmpp.tile([128, KD], dt.float32, tag="oinv")
                    nc.sync.dma_start(oinv[:], b_oinv[l])
                    dsm = tmpp.tile([128, KDI], dt.float32, tag="dsm")
                    nc.sync.dma_start(dsm[:], d_ssm[l])
                    wout = wwp.tile([128, KDI, 1024], dt.float8e4, tag="wout")
                    nc.sync.dma_start(wout[:], w_out[l])

                rms_x8(f"L{l}_rms", pssq)

                # --- in_proj fp8 DoubleRow, xs half first (j 0..15) ---
                with nc.named_scope(f"L{l}_inproj"):
                    wgs = []
                    for g in range(NG):
                        wg = wgp.tile([128, KD, 1024], dt.float8e4, tag="wg")
                        nc.sync.dma_start(wg[:], w_in[l][g])
                        wgs.append(wg)
                    for g in range(2):
                        wg = wgs[g]
                        for jj in range(GJ):
                            pt = mm_ps()
                            for t in range(KD // 2):
                                nc.tensor.matmul(
                                    pt[:],
                                    wg[:, 2 * t:2 * t + 2, jj * 128:(jj + 1) * 128],
                                    x8[:, 2 * t:2 * t + 2, :],
                                    start=(t == 0), stop=(t == KD // 2 - 1),
                                    perf_mode=DR)
                            j = g * GJ + jj
                            nc.scalar.activation(xs_bf[:, j, :], pt[:], AF.Silu,
                                                 bias=bcv[:, j:j + 1],
                                                 scale=cin[:, j:j + 1])

                if l == NL - 1:
                    # prefetch dense-MLP weights; DMA overlaps z/dt/y phases
                    dense_wg = []
                    for g in range(2):
                        wgd = dwgp.tile([128, KD, 1024], dt.bfloat16, tag="dwg",
                                        name=f"dense_wg{g}")
                        nc.sync.dma_start(wgd[:], w_d1[g])
                        dense_wg.append(wgd)

                # --- x_proj (needs only xs) -> dlo; s broadcast chain ---
                with nc.named_scope(f"L{l}_xproj"):
                    pdb = mm_ps(tag="small", bufs=2, shape=(DTR + 2 * N, BL))
                    for k in range(KDI):
                        nc.tensor.matmul(pdb[:], wxp[:, k, :], xs_bf[:, k, :],
                                         start=(k == 0), stop=(k == KDI - 1))
                    # dlo straight off PSUM (shortens the dt critical path)
                    nc.vector.tensor_copy(dlo_t[:DTR, :], pdb[:DTR, :])
                    nc.scalar.copy(dbl_sb[DTR:, :], pdb[DTR:, :])
                    nc.sync.dma_start(bm_t[:], dbl_sb[DTR:DTR + N, :])
                    nc.sync.dma_start(cm_t[:], dbl_sb[DTR + N:DTR + 2 * N, :])
                    nc.vector.tensor_mul(prod_b[:], bm_t[:], cm_t[:])
                    psdot = mm_ps(tag="small", bufs=2, shape=(1, BL))
                    nc.tensor.matmul(psdot[:], ones16_b[:], prod_b[:],
                                     start=True, stop=True)
                    nc.scalar.activation(s_row[:], psdot[:], AF.Copy, scale=CP)
                    psbc = mm_ps(tag="small", bufs=2)
                    nc.tensor.matmul(psbc[:], ones1_b[:], s_row[:],
                                     start=True, stop=True)
                    nc.vector.tensor_copy(s_bc[:], psbc[:])

                # --- z half with paired [128,1024] silu evacs; the ACT z/delta
                #     work overlaps the PE z/out GEMMs ---
                with nc.named_scope(f"L{l}_zproj"):
                    for g in range(2, NG):
                        wg = wgs[g]
                        for jj in range(GJ):
                            pt = mm_ps()
                            for t in range(KD // 2):
                                nc.tensor.matmul(
                                    pt[:],
                                    wg[:, 2 * t:2 * t + 2, jj * 128:(jj + 1) * 128],
                                    x8[:, 2 * t:2 * t + 2, :],
                                    start=(t == 0), stop=(t == KD // 2 - 1),
                                    perf_mode=DR)
                            zj = (g - 2) * GJ + jj
                            nc.scalar.activation(sz_bf[:, zj, :], pt[:], AF.Silu,
                                                 scale=cin[:, KDI + zj:KDI + zj + 1])
                            # q = xs*sz on gpsimd (overwrites sz)
                            nc.gpsimd.tensor_mul(sz_bf[:, zj, :],
                                                 xs_bf[:, zj, :],
                                                 sz_bf[:, zj, :])

                # --- dt matmul (K=65 incl bias row); softplus = Ln(Exp(u)+1);
                #     Exp/Ln/m1/p8 interleaved per k-pair so the first out_proj
                #     DR group starts ~2.5us after the first dt psum ---
                with nc.named_scope(f"L{l}_dt"):
                    for j in range(KDI):
                        pt = mm_ps()
                        nc.tensor.matmul(pt[:], wdt[:, j, :], dlo_t[:],
                                         start=True, stop=True)
                        nc.scalar.activation(delta_bf[:, j, :], pt[:], AF.Exp)
                        if j % 2 == 1:
                            nc.scalar.activation(delta_bf[:, j - 1:j + 1, :],
                                                 delta_bf[:, j - 1:j + 1, :],
                                                 AF.Ln, bias=1.0)
                            for k in (j - 1, j):
                                nc.vector.tensor_mul(delta_bf[:, k, :],
                                                     delta_bf[:, k, :], s_bc[:])
                                nc.vector.scalar_tensor_tensor(
                                    p8[:, k, :], delta_bf[:, k, :],
                                    dsm[:, k:k + 1], sz_bf[:, k, :],
                                    ALU.add, ALU.mult)

                # --- out_proj DR over p8; fused residual evac; next-layer
                #     squares alternate engines to avoid one serial chain ---
                with nc.named_scope(f"L{l}_y_out"):
                    if l < NL - 1:
                        pssq = mm_ps(tag="small", bufs=2, shape=(1, BL))
                    pouts = [mm_ps() for _ in range(KD // 2)]
                    for t in range(KDI // 2):
                        for j in range(KD // 2):
                            nc.tensor.matmul(
                                pouts[j][:],
                                wout[:, 2 * t:2 * t + 2, j * 128:(j + 1) * 128],
                                p8[:, 2 * t:2 * t + 2, :],
                                start=(t == 0), stop=(t == KDI // 2 - 1),
                                perf_mode=DR)

                    def evac_out(j, pt):
                        nc.vector.scalar_tensor_tensor(
                            xT[:, j, :], pt[:], oinv[:, j:j + 1], xT[:, j, :],
                            ALU.mult, ALU.add)
                        if l < NL - 1:
                            if j % 2 == 0:
                                nc.gpsimd.tensor_mul(sq_bf[:, j, :], xT[:, j, :],
                                                     xT[:, j, :])
                            else:
                                nc.scalar.activation(sq_bf[:, j, :], xT[:, j, :],
                                                     AF.Square)
                        else:
                            if j % 2 == 0:
                                nc.scalar.copy(sq_bf[:, j, :], xT[:, j, :])
                            else:
                                nc.vector.tensor_copy(sq_bf[:, j, :], xT[:, j, :])

                    for j in range(KD // 2):
                        evac_out(j, pouts[j])
                    for j in range(KD // 2, KD):
                        pt = mm_ps()
                        for t in range(KDI // 2):
                            nc.tensor.matmul(
                                pt[:],
                                wout[:, 2 * t:2 * t + 2, j * 128:(j + 1) * 128],
                                p8[:, 2 * t:2 * t + 2, :],
                                start=(t == 0), stop=(t == KDI // 2 - 1),
                                perf_mode=DR)
                        evac_out(j, pt)
                    if l < NL - 1:
                        # stats after all out matmuls (in-order PE queue:
                        # these wait on sq without blocking out GEMMs)
                        for j in range(KD):
                            nc.tensor.matmul(pssq[:], ones_bf[:], sq_bf[:, j, :],
                                             start=(j == 0), stop=(j == KD - 1))

            # ======== dense MLP (bf16): x -> g1 -> out ========
            # sq_bf holds the bf16 copy of the final residual.
            with nc.named_scope("dense_mlp"):
                for g in range(2):
                    wg = dense_wg[g]
                    for jj in range(GJ):
                        pt = mm_ps()
                        for k in range(KD):
                            nc.tensor.matmul(pt[:], wg[:, k, jj * 128:(jj + 1) * 128],
                                             sq_bf[:, k, :],
                                             start=(k == 0), stop=(k == KD - 1))
                        j = g * GJ + jj
                        nc.scalar.activation(xs_bf[:, j, :], pt[:], AF.Tanh,
                                             bias=bd1_sb[:, j:j + 1])
                for j in range(2):
                    pt = mm_ps()
                    for k in range(16):
                        nc.tensor.matmul(pt[:], wd2_sb[:, k, j * 128:(j + 1) * 128],
                                         xs_bf[:, k, :], start=(k == 0),
                                         stop=(k == 15))
                    nc.scalar.activation(out_sb[:, j, :], pt[:], AF.Tanh,
                                         bias=bd2_sb[:, j:j + 1])
                    nc.gpsimd.dma_start(out_d[j], out_sb[:, j, :])

    nc.compile()
    return nc


def _run(inputs, trace=False, trace_kwargs=None):
    if "nc" not in _cache:
        _cache["nc"] = _build()
    nc = _cache["nc"]
    p, xc = _host_pack(inputs)
    in_maps = []
    for c in range(NCORES):
        m = dict(p)
        m["x_in"] = xc[c]
        in_maps.append(m)

    from concourse.bass_utils import run_bass_kernel_spmd
    kw = {}
    if trace:
        kw.update(trace=True, trace_cores=[0], trace_kwargs=trace_kwargs or {})
    res = run_bass_kernel_spmd(nc, in_maps, core_ids=list(range(NCORES)), **kw)

    # assemble: per core out [2, 128, BL] -> out^T [256, BL] -> [BL, 256]
    full = np.empty((B, OUT), np.float32)
    for c in range(NCORES):
        o = res.results[c]["out"].reshape(OUT, BL)
        full[c * BL:(c + 1) * BL] = o.T
    return full.reshape(-1), res


def kernel(**inputs):
    out, _ = _run(inputs, trace=False)
    return out
